# revision 15
# baseline (speedup 1.0000x reference)
"""LogicLayer Trainium2 kernel: out = c0 + c1*x[:,ia] + c2*x[:,ib] + c3*x[:,ia]*x[:,ib]
with coef = softmax(weights) @ OP_COEFFS (softmax+projection computed on-device).

Sharding: out_dim (neuron) split across 8 cores, 2048 neurons each; x is
replicated, staged host-side pre-transposed to xT [in_dim, batch] fp16 so each
neuron's two input columns are contiguous 4 KB rows. Per core: SWDGE dma_gather
of a/b rows (fp16), ACT computes u = c3*b + c1, DVE computes w = c2*b + c0
(scalar_tensor_tensor with a broadcast c0 operand), then out = a*u + w in the
[neuron, batch] domain. Output is written transposed [neuron, batch] fp16 and
the host re-transposes/upcasts. No PE transposes, no PSUM, no DRAM scratch.
"""
import sys

sys.path.insert(0, "/opt/trn_rl_repo")
import numpy as np

import concourse.bass as bass  # noqa: F401
import concourse.bacc as bacc
from concourse import mybir
from concourse.bass_utils import run_bass_kernel_spmd

_OP_COEFFS = np.array([
    [0., 0., 0., 0.], [0., 0., 0., 1.], [0., 1., 0., -1.], [0., 1., 0., 0.],
    [0., 0., 1., -1.], [0., 0., 1., 0.], [0., 1., 1., -2.], [0., 1., 1., -1.],
    [1., -1., -1., 1.], [1., -1., -1., 2.], [1., 0., -1., 0.], [1., 0., -1., 1.],
    [1., -1., 0., 0.], [1., -1., 0., 1.], [1., 0., 0., -1.], [1., 0., 0., 0.],
], dtype=np.float32)

BATCH, IN_DIM, OUT_DIM = 2048, 16384, 16384
NCORES = 8
N = OUT_DIM // NCORES        # 2048 neurons per core
B = BATCH                    # full batch per core
NCH = N // 128               # 16 chunks of 128 neurons
NQTR = 4                     # pipeline quarters
QC = NCH // NQTR             # 4 chunks per quarter
NI = N // NQTR               # 512 gathered idxs per dma_gather call
F32 = mybir.dt.float32
F16 = mybir.dt.float16
I16 = mybir.dt.int16
AX = mybir.AxisListType.X
IDENT = mybir.ActivationFunctionType.Identity
EXP = mybir.ActivationFunctionType.Exp
MULT = mybir.AluOpType.mult
ADD = mybir.AluOpType.add

_cached = {}


def build_nc():
    nc = bacc.Bacc("TRN2", target_bir_lowering=False, num_swdge_queues=4)
    xt = nc.declare_dram_parameter("xt", [IN_DIM, B], F16, isOutput=False)
    wc_in = nc.declare_dram_parameter("wc", [128, NCH * 16], F32, isOutput=False)
    opc_in = nc.declare_dram_parameter("opc", [128, 4 * NCH * 16], F32, isOutput=False)
    ia_in = nc.declare_dram_parameter("ia", [128, N // 16], I16, isOutput=False)
    ib_in = nc.declare_dram_parameter("ib", [128, N // 16], I16, isOutput=False)
    out = nc.declare_dram_parameter("out", [N, B], F16, isOutput=True)
    out_r = out.ap().rearrange("(c p) b -> p c b", p=128)  # [p, c, b] = out[c*128+p, b]

    from contextlib import ExitStack
    es = ExitStack()
    sb = lambda n, shape, dt=F32: es.enter_context(nc.sbuf_tensor(n, shape, dt))
    sem = lambda n: es.enter_context(nc.semaphore(n))
    iat = sb("iat", [128, N // 16], I16); ibt = sb("ibt", [128, N // 16], I16)
    wt = sb("wt", [128, NCH * 16]); opcf = sb("opcf", [128, 4, NCH * 16])
    ssum = sb("ssum", [128, NCH]); rinv = sb("rinv", [128, NCH])
    ckn = sb("ckn", [128, NCH]); ck = sb("ck", [128, 4, NCH])
    m3 = sb("m3", [128, NCH * 16])
    ga = sb("ga", [128, NCH, B], F16); gb = sb("gb", [128, NCH, B], F16)
    uf = sb("uf", [128, 2, QC, B], F16); wf = sb("wf", [128, 2, QC, B], F16)
    ldi = sem("ldi"); ldw = sem("ldw")
    gsa = [sem(f"gsa{q}") for q in range(NQTR)]
    gsb = [sem(f"gsb{q}") for q in range(NQTR)]
    cfA = sem("cfA"); cfE = sem("cfE"); ua = sem("ua"); pd = sem("pd")
    sod = sem("sod"); outd = sem("outd"); ccs = sem("ccs"); wq = sem("wq")
    with es, nc.Block() as block:

        @block.sync
        def _(sync):
            sync.dma_start(iat[:], ia_in[:]).then_inc(ldi, 16)
            sync.dma_start(ibt[:], ib_in[:]).then_inc(ldi, 16)
            sync.dma_start(wt[:], wc_in[:]).then_inc(ldw, 16)
            sync.dma_start(
                opcf.ap().rearrange("p a b -> p (a b)"), opc_in[:]
            ).then_inc(ldw, 16)
            for q in range(NQTR):
                sync.wait_ge(sod, q + 1)
                sync.dma_start(
                    out_r[:, q * QC:(q + 1) * QC, :], ga[:, q * QC:(q + 1) * QC, :]
                ).then_inc(outd, 16)

        @block.gpsimd
        def _(gp):
            gp.wait_ge(ldi, 32)
            for q in range(NQTR):
                gp.dma_gather(
                    ga[:, q * QC:(q + 1) * QC, :], xt[:],
                    iat[:, q * (NI // 16):(q + 1) * (NI // 16)],
                    num_idxs=NI, num_idxs_reg=NI, elem_size=B,
                    single_packet=False, queue_num=(2 * q) % 4,
                ).then_inc(gsa[q], 16)
                gp.dma_gather(
                    gb[:, q * QC:(q + 1) * QC, :], xt[:],
                    ibt[:, q * (NI // 16):(q + 1) * (NI // 16)],
                    num_idxs=NI, num_idxs_reg=NI, elem_size=B,
                    single_packet=False, queue_num=(2 * q + 1) % 4,
                ).then_inc(gsb[q], 16)

        @block.scalar
        def _(act):
            act.wait_ge(ldw, 32)
            act.activation(wt[:], wt[:], EXP).then_inc(cfA, 1)
            act.wait_ge(cfE, 1)
            for q in range(NQTR):
                act.wait_ge(gsb[q], 16)
                if q >= 2:
                    act.wait_ge(pd, q - 1)          # uf[q%2] free
                for j in range(QC):
                    c = q * QC + j
                    act.activation(                  # u = c3*b + c1
                        uf[:, q % 2, j, :], gb[:, c, :], IDENT,
                        bias=ck[:, 1, c:c + 1], scale=ck[:, 3, c:c + 1],
                    ).then_inc(ua, 1)

        @block.vector
        def _(vec):
            # coef chain: ck[p, k, c] = coef_k(neuron c*128+p). Fully
            # serialized via ccs — DVE does not interlock same-engine RAW.
            nedge = [0]

            def edge(inst):
                nedge[0] += 1
                inst.then_inc(ccs, 1)
                vec.wait_ge(ccs, nedge[0])

            vec.wait_ge(cfA, 1)
            e3 = wt.ap().rearrange("p (a b) -> p a b", b=16)
            m3r = m3.ap().rearrange("p (a b) -> p a b", b=16)
            edge(vec.reduce_sum(ssum[:], e3, axis=AX))
            edge(vec.reciprocal(rinv[:], ssum[:]))
            vec.wait_ge(ldw, 32)
            for k in range(4):
                o3c = opcf[:, k, :].rearrange("p (a b) -> p a b", b=16)
                edge(vec.tensor_mul(m3r, e3, o3c))
                edge(vec.reduce_sum(ckn[:], m3r, axis=AX))
                inst = vec.tensor_mul(ck[:, k, :], ckn[:], rinv[:])
                if k == 3:
                    inst.then_inc(cfE, 1)
                else:
                    edge(inst)
            vec.wait_ge(cfE, 1)
            for q in range(NQTR):
                vec.wait_ge(gsb[q], 16)
                if q >= 2:
                    vec.wait_ge(sod, q - 1)             # wf[q%2] free
                for j in range(QC):
                    c = q * QC + j
                    c0b = ck[:, 0, c:c + 1].to_broadcast([128, B])
                    inst = vec.scalar_tensor_tensor(    # w = c2*b + c0
                        wf[:, q % 2, j, :], gb[:, c, :], ck[:, 2, c:c + 1],
                        c0b, MULT, ADD,
                    )
                    if j == QC - 1:
                        inst.then_inc(wq, 1)            # gb_q reads done (DVE)
                vec.wait_ge(wq, q + 1)
                vec.wait_ge(ua, QC * (q + 1))           # gb_q reads done (ACT)
                vec.wait_ge(gsa[q], 16)
                qs = slice(q * QC, (q + 1) * QC)
                vec.tensor_mul(gb[:, qs, :], ga[:, qs, :], uf[:, q % 2]).then_inc(pd, 1)
                vec.wait_ge(pd, q + 1)                  # prod drained before so reads it
                vec.tensor_add(ga[:, qs, :], gb[:, qs, :], wf[:, q % 2]).then_inc(sod, 1)

    nc.compile()
    return nc


def wrap_idx(vals):
    """Per-call wrapped int16 tables, concatenated: [128, N//16]."""
    cols = []
    for q in range(NQTR):
        v = np.asarray(vals[q * NI:(q + 1) * NI])
        arr = v.reshape(NI // 16, 16).T.astype(np.int16)   # [16, NI//16]
        cols.append(np.tile(arr, (8, 1)))                  # [128, NI//16]
    return np.ascontiguousarray(np.concatenate(cols, axis=1))


def kernel(x, idx_a, idx_b, weights, trace=False):
    x = np.asarray(x, dtype=np.float32)
    weights = np.asarray(weights, dtype=np.float32)
    idx_a = np.asarray(idx_a)
    idx_b = np.asarray(idx_b)

    if "nc" not in _cached:
        _cached["nc"] = build_nc()
    nc = _cached["nc"]

    xt = np.ascontiguousarray(x.astype(np.float16).T)  # [IN_DIM, BATCH] fp16
    opc_row = np.repeat(_OP_COEFFS.T[:, None, :], NCH, axis=1).reshape(4 * NCH * 16)
    opc = np.ascontiguousarray(
        np.broadcast_to(opc_row[None, :], (128, 4 * NCH * 16))
    ).astype(np.float32)

    in_maps = []
    for k in range(NCORES):
        s = slice(k * N, (k + 1) * N)
        wc = np.ascontiguousarray(
            weights[s].reshape(NCH, 128, 16).transpose(1, 0, 2).reshape(128, NCH * 16)
        )
        in_maps.append({
            "xt": xt, "wc": wc, "opc": opc,
            "ia": wrap_idx(idx_a[s]), "ib": wrap_idx(idx_b[s]),
        })
    res = run_bass_kernel_spmd(nc, in_maps, core_ids=list(range(NCORES)), trace=trace)
    out = np.empty((BATCH, OUT_DIM), dtype=np.float32)
    for k in range(NCORES):
        out[:, k * N:(k + 1) * N] = res.results[k]["out"].T
    kernel.last_exec_time_ns = res.exec_time_ns
    return out


kernel.last_exec_time_ns = None
